# revision 1
# baseline (speedup 1.0000x reference)
"""ConvFace GNN message-passing kernel for Trainium2 (8 NeuronCores).

Computation (per batch b, pooled face f):
  cat   = [fea[:, pool_idx[f]], fea[:, ring_n[b,f,0..15]]]           # [C, 17]
  keyv  = Wk @ cat[:,0] + bk
  att_k = softmax_k( keyv . (Wq @ cat[:,k] + bq) / sqrt(128) )
        = softmax_k( g . cat[:,k] / sqrt(128) ),  g = Wq^T keyv      # bq drops
  agg   = cat @ att
  y     = Wc @ agg (+ bc)  -> BatchNorm(train stats over (b,f)) -> ReLU
bc shifts only the BN mean, so it cancels; bq only adds a k-constant to the
logits, so it cancels in softmax.  pos_embed is all-zero / unused.

Sharding: core c <- (batch b = c//2, face half h = c%2), 5000 faces each.
fea is passed per-batch transposed to [F, C] so each neighbor gather is a
contiguous 256 B row fetched with one dma_gather descriptor.  BN statistics
are AllReduce'd across the 8 cores inside the kernel.
"""

import numpy as np

import concourse.bass as bass
import concourse.bacc as bacc
import concourse.mybir as mybir
import concourse.tile as tile
from concourse import library_config
from concourse.bass_utils import run_bass_kernel_spmd

AF = mybir.ActivationFunctionType
ALU = mybir.AluOpType
F32 = mybir.dt.float32
I16 = mybir.dt.int16
I32 = mybir.dt.int32

# full-problem constants
B, C, F, FP, K, O = 4, 64, 20000, 10000, 16, 128
K1 = K + 1
NCORES = 8
SQRT_DK = float(np.sqrt(128.0))
BN_EPS = 1e-5


def build_nc(
    n_faces=F,          # rows of fea_t (gather source)
    T=40,               # face tiles of 128 per core
    TPC=4,              # tiles per dma_gather chunk
    fpc_valid=5000,     # valid faces per core (<= T*128)
    ntot=B * FP,        # global BN sample count
    num_devices=NCORES,
    ks_dve=11,          # k in [0, ks_dve) of the agg product on DVE, rest on GPSIMD
):
    assert T % TPC == 0
    nchunk = T // TPC
    NI = TPC * K1 * 128          # indices per gather
    NIW = NI // 16               # wrapped idx columns per gather

    nc = bacc.Bacc(trn_type="TRN2", num_devices=num_devices)

    fea_t = nc.dram_tensor("fea_t", [n_faces, C], F32, kind="ExternalInput")
    idx32 = nc.dram_tensor("idx32", [128, T * K1], I32, kind="ExternalInput")
    a_aug = nc.dram_tensor("a_aug", [C + 1, C], F32, kind="ExternalInput")
    wct = nc.dram_tensor("wct", [C, O], F32, kind="ExternalInput")
    gamma = nc.dram_tensor("gamma", [O, 1], F32, kind="ExternalInput")
    beta = nc.dram_tensor("beta", [O, 1], F32, kind="ExternalInput")
    ident = nc.dram_tensor("ident", [128, 128], F32, kind="ExternalInput")
    y_out = nc.dram_tensor("y_out", [O, fpc_valid], F32, kind="ExternalOutput")
    if num_devices > 1:
        cc_in = nc.dram_tensor("cc_in", [O, 2], F32, kind="Internal")
        cc_out = nc.dram_tensor(
            "cc_out", [O, 2], F32, kind="Internal",
            addr_space="Shared" if num_devices > 4 else "Local",
        )

    with tile.TileContext(nc) as tc:
        with (
            tc.tile_pool(name="singles", bufs=1) as singles,
            tc.tile_pool(name="gd", bufs=2) as gd_pool,
            tc.tile_pool(name="mid", bufs=2) as mid,
            tc.tile_pool(name="prod", bufs=2) as prod_pool,
            tc.tile_pool(name="prod2", bufs=2) as prod2_pool,
            tc.tile_pool(name="small", bufs=3) as small,
            tc.tile_pool(name="sq", bufs=2) as sq_pool,
            tc.tile_pool(name="pst", bufs=3, space="PSUM") as pst,
            tc.tile_pool(name="pgf", bufs=2, space="PSUM") as pgf,
            tc.tile_pool(name="py", bufs=2, space="PSUM") as py,
        ):

            # constants / persistent buffers
            idx_sb = singles.tile([128, T * K1], I32)
            nc.sync.dma_start(out=idx_sb[:], in_=idx32[:])
            a_sb = singles.tile([C + 1, C], F32)
            nc.sync.dma_start(out=a_sb[:], in_=a_aug[:])
            wct_sb = singles.tile([C, O], F32)
            nc.sync.dma_start(out=wct_sb[:], in_=wct[:])
            gamma_sb = singles.tile([O, 1], F32)
            nc.sync.dma_start(out=gamma_sb[:], in_=gamma[:])
            beta_sb = singles.tile([O, 1], F32)
            nc.sync.dma_start(out=beta_sb[:], in_=beta[:])
            ident_sb = singles.tile([128, 128], F32)
            nc.sync.dma_start(out=ident_sb[:], in_=ident[:])

            xsT_aug = singles.tile([C + 1, 128], F32)  # row C is constant 1.0
            nc.vector.memset(xsT_aug[C : C + 1, :], 1.0)
            zero_t = singles.tile([128, 1], F32)
            nc.vector.memset(zero_t[:], 0.0)
            eps_t = singles.tile([O, 1], F32)
            nc.vector.memset(eps_t[:], BN_EPS)

            ybuf = singles.tile([128, T * 128], F32)
            sums = singles.tile([O, T], F32)
            sqs = singles.tile([O, T], F32)

            for g in range(nchunk):
                gd = gd_pool.tile([128, TPC * K1, C], F32)
                nc.gpsimd.indirect_dma_start(
                    out=gd[:],
                    out_offset=None,
                    in_=fea_t[:],
                    in_offset=bass.IndirectOffsetOnAxis(
                        ap=idx_sb[:, g * TPC * K1 : (g + 1) * TPC * K1], axis=0
                    ),
                )
                for s in range(TPC):
                    t = g * TPC + s
                    nv = min(128, fpc_valid - t * 128)
                    if nv <= 0:
                        break
                    cat = gd[:, s * K1 : (s + 1) * K1, :]  # [128, K1, C]

                    # ---- G = (Wq^T Wk xs + Wq^T bk)/sqrt(dk), face-major ----
                    xsT_psum = pst.tile([C, 128], F32, tag="pst")
                    nc.tensor.transpose(xsT_psum[:], cat[:, 0, :], ident_sb[:])
                    nc.scalar.activation(xsT_aug[0:C, :], xsT_psum[:], AF.Copy)
                    gt_psum = pst.tile([C, 128], F32, tag="pst")
                    nc.tensor.matmul(
                        gt_psum[:], lhsT=a_sb[:], rhs=xsT_aug[:], start=True, stop=True
                    )
                    gt_sb = mid.tile([C, 128], F32, tag="gt")
                    nc.scalar.activation(gt_sb[:], gt_psum[:], AF.Copy)
                    gf_psum = pgf.tile([128, C], F32)
                    nc.tensor.transpose(gf_psum[:], gt_sb[:], ident_sb[0:C, 0:C])
                    gf_sb = mid.tile([128, C], F32, tag="gf")
                    nc.scalar.activation(gf_sb[:], gf_psum[:], AF.Copy)

                    # ---- logits[f,k] = sum_c G[f,c] * cat[f,k,c] (pre-scaled) ----
                    prod = prod_pool.tile([128, K1, C], F32)
                    gf_b = gf_sb[:].unsqueeze(1).to_broadcast([128, K1, C])
                    nc.vector.tensor_tensor(
                        out=prod[:], in0=cat, in1=gf_b, op=ALU.mult
                    )
                    logits = small.tile([128, K1], F32, tag="logits")
                    nc.vector.tensor_reduce(
                        out=logits[:], in_=prod[:], axis=mybir.AxisListType.X,
                        op=ALU.add,
                    )

                    # ---- softmax over k (logits are small; skip max-sub) ----
                    attu = small.tile([128, K1], F32, tag="attu")
                    ssum = small.tile([128, 1], F32, tag="ssum")
                    nc.scalar.activation(attu[:], logits[:], AF.Exp,
                                         bias=zero_t[:], accum_out=ssum[:])
                    rinv = small.tile([128, 1], F32, tag="rinv")
                    nc.vector.reciprocal(rinv[:], ssum[:])
                    att = small.tile([128, K1], F32, tag="att")
                    nc.vector.tensor_scalar(
                        out=att[:], in0=attu[:], scalar1=rinv[:], scalar2=None,
                        op0=ALU.mult,
                    )

                    # ---- agg[f,c] = sum_k att[f,k] * cat[f,k,c] ----
                    prod2 = prod2_pool.tile([128, K1, C], F32)
                    att_b = att[:].unsqueeze(2).to_broadcast([128, K1, C])
                    ks = min(ks_dve, K1)
                    nc.vector.tensor_tensor(
                        out=prod2[:, 0:ks, :], in0=cat[:, 0:ks, :],
                        in1=att_b[:, 0:ks, :], op=ALU.mult,
                    )
                    if ks < K1:
                        nc.gpsimd.tensor_tensor(
                            out=prod2[:, ks:K1, :], in0=cat[:, ks:K1, :],
                            in1=att_b[:, ks:K1, :], op=ALU.mult,
                        )
                    agg = mid.tile([128, C], F32, tag="agg")
                    nc.vector.tensor_reduce(
                        out=agg[:], in_=prod2[:].rearrange("p k c -> p c k"),
                        axis=mybir.AxisListType.X, op=ALU.add,
                    )

                    # ---- y = Wc @ agg  (channel-major via PE transpose) ----
                    aggT_psum = pst.tile([C, 128], F32, tag="pst")
                    nc.tensor.transpose(aggT_psum[:], agg[:], ident_sb[:])
                    aggT_sb = mid.tile([C, 128], F32, tag="aggT")
                    nc.scalar.activation(aggT_sb[:], aggT_psum[:], AF.Copy)
                    y_psum = py.tile([O, 128], F32)
                    nc.tensor.matmul(
                        y_psum[:], lhsT=wct_sb[:], rhs=aggT_sb[:], start=True,
                        stop=True,
                    )

                    # ---- stash y + BN partial sums ----
                    nc.scalar.activation(
                        ybuf[:, t * 128 : t * 128 + nv], y_psum[:, 0:nv], AF.Copy,
                        accum_out=sums[:, t : t + 1],
                    )
                    sq_scr = sq_pool.tile([O, 128], F32)
                    nc.scalar.activation(
                        sq_scr[:, 0:nv], y_psum[:, 0:nv], AF.Square,
                        bias=zero_t[:], accum_out=sqs[:, t : t + 1],
                    )

            # ---- global BN stats ----
            stats_l = small.tile([O, 2], F32, tag="stats")
            nc.vector.tensor_reduce(
                out=stats_l[:, 0:1], in_=sums[:], axis=mybir.AxisListType.X, op=ALU.add
            )
            nc.vector.tensor_reduce(
                out=stats_l[:, 1:2], in_=sqs[:], axis=mybir.AxisListType.X, op=ALU.add
            )
            gst = small.tile([O, 2], F32, tag="gst")
            if num_devices > 1:
                nc.sync.dma_start(out=cc_in[:], in_=stats_l[:])
                nc.gpsimd.collective_compute(
                    "AllReduce",
                    ALU.add,
                    replica_groups=[list(range(num_devices))],
                    ins=[cc_in[:]],
                    outs=[cc_out[:]],
                )
                nc.sync.dma_start(out=gst[:], in_=cc_out[:])
            else:
                nc.vector.tensor_copy(out=gst[:], in_=stats_l[:])

            mean = small.tile([O, 1], F32, tag="mean")
            nc.vector.tensor_scalar_mul(mean[:], gst[:, 0:1], 1.0 / ntot)
            e2 = small.tile([O, 1], F32, tag="e2")
            nc.vector.tensor_scalar_mul(e2[:], gst[:, 1:2], 1.0 / ntot)
            negvar = small.tile([O, 1], F32, tag="negvar")
            nc.vector.scalar_tensor_tensor(
                out=negvar[:], in0=mean[:], scalar=mean[:], in1=e2[:],
                op0=ALU.mult, op1=ALU.subtract,
            )
            sd = small.tile([O, 1], F32, tag="sd")
            nc.scalar.activation(sd[:], negvar[:], AF.Sqrt, bias=eps_t[:], scale=-1.0)
            rstd = small.tile([O, 1], F32, tag="rstd")
            nc.vector.reciprocal(rstd[:], sd[:])
            scale_v = small.tile([O, 1], F32, tag="scale_v")
            nc.vector.tensor_tensor(
                out=scale_v[:], in0=rstd[:], in1=gamma_sb[:], op=ALU.mult
            )
            negshift = small.tile([O, 1], F32, tag="negshift")
            nc.vector.scalar_tensor_tensor(
                out=negshift[:], in0=mean[:], scalar=scale_v[:], in1=beta_sb[:],
                op0=ALU.mult, op1=ALU.subtract,
            )
            shift = small.tile([O, 1], F32, tag="shift")
            nc.vector.tensor_scalar_mul(shift[:], negshift[:], -1.0)

            # ---- final: relu((y - mean) * rstd * gamma + beta) ----
            for t in range(T):
                nv = min(128, fpc_valid - t * 128)
                if nv <= 0:
                    break
                sl = ybuf[:, t * 128 : t * 128 + nv]
                nc.scalar.activation(
                    sl, sl, AF.Relu, bias=shift[:], scale=scale_v[:]
                )
            nc.sync.dma_start(out=y_out[:], in_=ybuf[:, 0:fpc_valid])

    nc.compile()
    return nc


def prep_idx(cat_idx, T, TPC):
    """cat_idx [fpc_valid, K1] int -> int32 [128, T*K1]; face tile t of 128
    faces occupies columns [t*K1, (t+1)*K1): idx[p, t*K1+k] = cat_idx[t*128+p, k]."""
    fpp = T * 128
    pad = fpp - cat_idx.shape[0]
    ci = np.concatenate(
        [cat_idx, np.zeros((pad, K1), cat_idx.dtype)], 0
    ) if pad else cat_idx
    return np.ascontiguousarray(
        ci.reshape(T, 128, K1).transpose(1, 0, 2).reshape(128, T * K1)
    ).astype(np.int32)


def prep_weights(Wk, bk, Wq, bq, Wc, gamma, beta):
    Wk = np.asarray(Wk, np.float64)
    Wq = np.asarray(Wq, np.float64)
    bk = np.asarray(bk, np.float64)
    a_mat = (Wk.T @ Wq) / SQRT_DK                 # [c, j]
    u = (Wq.T @ bk) / SQRT_DK                     # [j]
    a_aug = np.concatenate([a_mat, u[None, :]], 0).astype(np.float32)  # [C+1, C]
    wct = np.ascontiguousarray(np.asarray(Wc, np.float32).T)           # [C, O]
    g = np.asarray(gamma, np.float32).reshape(O, 1).copy()
    b = np.asarray(beta, np.float32).reshape(O, 1).copy()
    ident = np.eye(128, dtype=np.float32)
    return a_aug, wct, g, b, ident


_T, _TPC = 40, 4
_FPC = FP // 2


def prepare(fea, ring_n, pool_idx, pos_embed=None, Wk=None, bk=None, Wq=None,
            bq=None, Wc=None, bc=None, gamma=None, beta=None):
    """Host-side sharding/layout prep. Returns (nc, in_maps)."""
    fea = np.asarray(fea, np.float32)
    ring_n = np.asarray(ring_n)
    pool_idx = np.asarray(pool_idx)

    T, TPC, fpc = _T, _TPC, _FPC
    a_aug, wct, g_np, b_np, ident = prep_weights(Wk, bk, Wq, bq, Wc, gamma, beta)
    fea_t = np.ascontiguousarray(fea.transpose(0, 2, 1))  # [B, F, C]

    in_maps = []
    for c in range(NCORES):
        b, h = c // 2, c % 2
        cat_idx = np.concatenate(
            [pool_idx[h * fpc : (h + 1) * fpc, None], ring_n[b, h * fpc : (h + 1) * fpc]],
            axis=1,
        )
        in_maps.append(
            {
                "fea_t": fea_t[b],
                "idx32": prep_idx(cat_idx, T, TPC),
                "a_aug": a_aug,
                "wct": wct,
                "gamma": g_np,
                "beta": b_np,
                "ident": ident,
            }
        )

    nc = build_nc(n_faces=F, T=T, TPC=TPC, fpc_valid=fpc, ntot=B * FP,
                  num_devices=NCORES)
    return nc, in_maps


def assemble(per_core_outs):
    """per_core_outs: list of {'y_out': [O, FPC]} -> full [B, O, FP]."""
    out = np.empty((B, O, FP), np.float32)
    for c in range(NCORES):
        b, h = c // 2, c % 2
        out[b, :, h * _FPC : (h + 1) * _FPC] = per_core_outs[c]["y_out"]
    return out


def kernel(fea, ring_n, pool_idx, pos_embed=None, Wk=None, bk=None, Wq=None,
           bq=None, Wc=None, bc=None, gamma=None, beta=None):
    nc, in_maps = prepare(fea, ring_n, pool_idx, pos_embed, Wk, bk, Wq, bq,
                          Wc, bc, gamma, beta)
    res = run_bass_kernel_spmd(nc, in_maps, core_ids=list(range(NCORES)))
    return assemble(res.results)


# ---------------------------------------------------------------------------
# v2: host-side gather fallback (the terminal rejects pool indirect DMAs).
# Device does all the per-face math from pre-gathered cat tiles; BatchNorm
# statistics and the final affine+ReLU are applied on the host from exact y.
# ---------------------------------------------------------------------------

def build_nc_v2(T=_T, fpc_valid=_FPC, num_devices=NCORES):
    nc = bacc.Bacc(trn_type="TRN2", num_devices=num_devices)
    cat_in = nc.dram_tensor("cat_in", [T, 128, K1 * C], F32, kind="ExternalInput")
    a_aug = nc.dram_tensor("a_aug", [C + 1, C], F32, kind="ExternalInput")
    wct = nc.dram_tensor("wct", [C, O], F32, kind="ExternalInput")
    ident = nc.dram_tensor("ident", [128, 128], F32, kind="ExternalInput")
    y_out = nc.dram_tensor("y_out", [O, fpc_valid], F32, kind="ExternalOutput")

    with tile.TileContext(nc) as tc:
        with (
            tc.tile_pool(name="singles", bufs=1) as singles,
            tc.tile_pool(name="gd", bufs=3) as gd_pool,
            tc.tile_pool(name="mid", bufs=2) as mid,
            tc.tile_pool(name="prod", bufs=2) as prod_pool,
            tc.tile_pool(name="prod2", bufs=2) as prod2_pool,
            tc.tile_pool(name="small", bufs=3) as small,
            tc.tile_pool(name="pst", bufs=3, space="PSUM") as pst,
            tc.tile_pool(name="pgf", bufs=2, space="PSUM") as pgf,
            tc.tile_pool(name="py", bufs=2, space="PSUM") as py,
        ):
            a_sb = singles.tile([C + 1, C], F32)
            nc.sync.dma_start(out=a_sb[:], in_=a_aug[:])
            wct_sb = singles.tile([C, O], F32)
            nc.sync.dma_start(out=wct_sb[:], in_=wct[:])
            ident_sb = singles.tile([128, 128], F32)
            nc.sync.dma_start(out=ident_sb[:], in_=ident[:])
            xsT_aug = singles.tile([C + 1, 128], F32)
            nc.vector.memset(xsT_aug[C : C + 1, :], 1.0)
            zero_t = singles.tile([128, 1], F32)
            nc.vector.memset(zero_t[:], 0.0)
            ybuf = singles.tile([128, T * 128], F32)

            for t in range(T):
                catf = gd_pool.tile([128, K1 * C], F32)
                nc.sync.dma_start(out=catf[:], in_=cat_in[t])
                cat = catf[:].rearrange("p (k c) -> p k c", k=K1)

                xsT_psum = pst.tile([C, 128], F32, tag="pst")
                nc.tensor.transpose(xsT_psum[:], cat[:, 0, :], ident_sb[:])
                nc.scalar.activation(xsT_aug[0:C, :], xsT_psum[:], AF.Copy)
                gt_psum = pst.tile([C, 128], F32, tag="pst")
                nc.tensor.matmul(gt_psum[:], lhsT=a_sb[:], rhs=xsT_aug[:],
                                 start=True, stop=True)
                gt_sb = mid.tile([C, 128], F32, tag="gt")
                nc.scalar.activation(gt_sb[:], gt_psum[:], AF.Copy)
                gf_psum = pgf.tile([128, C], F32)
                nc.tensor.transpose(gf_psum[:], gt_sb[:], ident_sb[0:C, 0:C])
                gf_sb = mid.tile([128, C], F32, tag="gf")
                nc.scalar.activation(gf_sb[:], gf_psum[:], AF.Copy)

                prod = prod_pool.tile([128, K1, C], F32)
                gf_b = gf_sb[:].unsqueeze(1).to_broadcast([128, K1, C])
                nc.vector.tensor_tensor(out=prod[:], in0=cat, in1=gf_b, op=ALU.mult)
                logits = small.tile([128, K1], F32, tag="logits")
                nc.vector.tensor_reduce(out=logits[:], in_=prod[:],
                                        axis=mybir.AxisListType.X, op=ALU.add)
                attu = small.tile([128, K1], F32, tag="attu")
                nc.scalar.activation(attu[:], logits[:], AF.Exp, bias=zero_t[:])
                ssum = small.tile([128, 1], F32, tag="ssum")
                nc.vector.tensor_reduce(out=ssum[:], in_=attu[:],
                                        axis=mybir.AxisListType.X, op=ALU.add)
                rinv = small.tile([128, 1], F32, tag="rinv")
                nc.vector.reciprocal(rinv[:], ssum[:])
                att = small.tile([128, K1], F32, tag="att")
                nc.vector.tensor_scalar(out=att[:], in0=attu[:], scalar1=rinv[:],
                                        scalar2=None, op0=ALU.mult)

                prod2 = prod2_pool.tile([128, K1, C], F32)
                att_b = att[:].unsqueeze(2).to_broadcast([128, K1, C])
                nc.vector.tensor_tensor(out=prod2[:], in0=cat, in1=att_b,
                                        op=ALU.mult)
                agg = mid.tile([128, C], F32, tag="agg")
                nc.vector.tensor_reduce(out=agg[:],
                                        in_=prod2[:].rearrange("p k c -> p c k"),
                                        axis=mybir.AxisListType.X, op=ALU.add)
                aggT_psum = pst.tile([C, 128], F32, tag="pst")
                nc.tensor.transpose(aggT_psum[:], agg[:], ident_sb[:])
                aggT_sb = mid.tile([C, 128], F32, tag="aggT")
                nc.scalar.activation(aggT_sb[:], aggT_psum[:], AF.Copy)
                y_psum = py.tile([O, 128], F32)
                nc.tensor.matmul(y_psum[:], lhsT=wct_sb[:], rhs=aggT_sb[:],
                                 start=True, stop=True)
                nc.scalar.activation(ybuf[:, t * 128 : (t + 1) * 128], y_psum[:],
                                     AF.Copy)

            nc.sync.dma_start(out=y_out[:], in_=ybuf[:, 0:fpc_valid])
    nc.compile()
    return nc


_NC_V2 = None


def kernel(fea, ring_n, pool_idx, pos_embed=None, Wk=None, bk=None, Wq=None,
           bq=None, Wc=None, bc=None, gamma=None, beta=None):
    fea = np.asarray(fea, np.float32)
    ring_n = np.asarray(ring_n)
    pool_idx = np.asarray(pool_idx)
    T, fpc = _T, _FPC
    a_aug, wct, g_np, b_np, ident = prep_weights(Wk, bk, Wq, bq, Wc, gamma, beta)
    fea_t = np.ascontiguousarray(fea.transpose(0, 2, 1))  # [B, F, C]

    in_maps = []
    for c in range(NCORES):
        b, h = c // 2, c % 2
        ci = np.concatenate(
            [pool_idx[h * fpc : (h + 1) * fpc, None],
             ring_n[b, h * fpc : (h + 1) * fpc]], axis=1).astype(np.int64)
        pad = T * 128 - ci.shape[0]
        if pad:
            ci = np.concatenate([ci, np.zeros((pad, K1), np.int64)], 0)
        cat = fea_t[b][ci.reshape(-1)]                        # [T*128*K1, C]
        cat = cat.reshape(T, 128, K1 * C)
        in_maps.append({"cat_in": cat, "a_aug": a_aug, "wct": wct,
                        "ident": ident})

    global _NC_V2
    if _NC_V2 is None:
        _NC_V2 = build_nc_v2(T=T, fpc_valid=fpc, num_devices=NCORES)
    res = run_bass_kernel_spmd(_NC_V2, in_maps, core_ids=list(range(NCORES)))

    y = np.empty((B, O, FP), np.float32)
    for c in range(NCORES):
        b, h = c // 2, c % 2
        y[b, :, h * fpc : (h + 1) * fpc] = res.results[c]["y_out"]
    mean = y.mean(axis=(0, 2), keepdims=True)
    var = y.var(axis=(0, 2), keepdims=True)
    yn = (y - mean) / np.sqrt(var + BN_EPS)
    yn = yn * np.asarray(gamma, np.float32)[None, :, None] \
        + np.asarray(beta, np.float32)[None, :, None]
    return np.maximum(yn, 0.0)



# revision 2
# speedup vs baseline: 7.0416x; 7.0416x over previous
"""ConvFace GNN message-passing kernel for Trainium2 (8 NeuronCores).

Computation (per batch b, pooled face f):
  cat   = [fea[:, pool_idx[f]], fea[:, ring_n[b,f,0..15]]]           # [C, 17]
  keyv  = Wk @ cat[:,0] + bk
  att_k = softmax_k( keyv . (Wq @ cat[:,k] + bq) / sqrt(128) )
        = softmax_k( g . cat[:,k] / sqrt(128) ),  g = Wq^T keyv      # bq drops
  agg   = cat @ att
  y     = Wc @ agg (+ bc)  -> BatchNorm(train stats over (b,f)) -> ReLU
bc shifts only the BN mean, so it cancels; bq only adds a k-constant to the
logits, so it cancels in softmax.  pos_embed is all-zero / unused.

Sharding: core c <- (batch b = c//2, face half h = c%2), 5000 faces each.

The axon link to the NeuronCores moves ~40 MB/s each way, so the kernel is
link-bound: every core uploads only its own HALF of its batch's fea (f16,
1.28 MB) and the full per-batch gather table is rebuilt on device with a
pair-wise AllGather over NeuronLink.  Neighbor rows are then fetched on
device with per-partition indirect DMAs (128 rows per descriptor set; the
batched [128,K] offset form is broken on this terminal).  BN statistics are
AllReduce'd across the 8 cores on device; the final affine+ReLU result is
downloaded as f16.  The PJRT executable is built once and cached; the
donated output buffers are created on device instead of uploading zeros.
"""

import numpy as np

import concourse.bass as bass
import concourse.bacc as bacc
import concourse.mybir as mybir
import concourse.tile as tile

AF = mybir.ActivationFunctionType
ALU = mybir.AluOpType
F32 = mybir.dt.float32
F16 = mybir.dt.float16
I16 = mybir.dt.int16
I32 = mybir.dt.int32

# full-problem constants
B, C, F, FP, K, O = 4, 64, 20000, 10000, 16, 128
K1 = K + 1
NCORES = 8
SQRT_DK = float(np.sqrt(128.0))
BN_EPS = 1e-5

_T = 40                 # face tiles of 128 per core
_FPC = FP // 2          # valid faces per core


def build_nc(T=_T, fpc=_FPC, ndev=NCORES, ntot=B * FP):
    nc = bacc.Bacc(trn_type="TRN2", num_devices=ndev)

    fea_h = nc.dram_tensor("fea_h", [FP, C], F16, kind="ExternalInput")
    idx16 = nc.dram_tensor("idx16", [128, T * K1], I16, kind="ExternalInput")
    a_aug = nc.dram_tensor("a_aug", [C + 1, C], F32, kind="ExternalInput")
    wct = nc.dram_tensor("wct", [C, O], F32, kind="ExternalInput")
    gb = nc.dram_tensor("gb", [O, 2], F32, kind="ExternalInput")
    ident = nc.dram_tensor("ident", [128, 128], F32, kind="ExternalInput")
    y_out = nc.dram_tensor("y_out", [O, fpc], F16, kind="ExternalOutput")

    fea_loc = nc.dram_tensor("fea_loc", [FP, C], F16, kind="Internal")
    fea_full = nc.dram_tensor("fea_full", [F, C], F16, kind="Internal")
    cc_sin = nc.dram_tensor("cc_sin", [O, 2], F32, kind="Internal")
    cc_sout = nc.dram_tensor(
        "cc_sout", [O, 2], F32, kind="Internal", addr_space="Shared"
    )

    with tile.TileContext(nc) as tc:
        with (
            tc.tile_pool(name="singles", bufs=1) as singles,
            tc.tile_pool(name="gd", bufs=3) as gd_pool,
            tc.tile_pool(name="cat", bufs=2) as cat_pool,
            tc.tile_pool(name="mid", bufs=2) as mid,
            tc.tile_pool(name="prod", bufs=2) as prod_pool,
            tc.tile_pool(name="prod2", bufs=2) as prod2_pool,
            tc.tile_pool(name="small", bufs=3) as small,
            tc.tile_pool(name="sq", bufs=2) as sq_pool,
            tc.tile_pool(name="pst", bufs=3, space="PSUM") as pst,
            tc.tile_pool(name="pgf", bufs=2, space="PSUM") as pgf,
            tc.tile_pool(name="py", bufs=2, space="PSUM") as py,
        ):
            # rebuild the full per-batch gather table from the two halves
            nc.sync.dma_start(out=fea_loc[:], in_=fea_h[:])
            nc.gpsimd.collective_compute(
                "AllGather",
                ALU.bypass,
                replica_groups=[[0, 1], [2, 3], [4, 5], [6, 7]],
                ins=[fea_loc[:]],
                outs=[fea_full[:]],
            )

            # constants / persistent buffers
            idx_sb16 = singles.tile([128, T * K1], I16)
            nc.sync.dma_start(out=idx_sb16[:], in_=idx16[:])
            idx_sb = singles.tile([128, T * K1], I32)
            nc.vector.tensor_copy(out=idx_sb[:], in_=idx_sb16[:])
            a_sb = singles.tile([C + 1, C], F32)
            nc.sync.dma_start(out=a_sb[:], in_=a_aug[:])
            wct_sb = singles.tile([C, O], F32)
            nc.sync.dma_start(out=wct_sb[:], in_=wct[:])
            gb_sb = singles.tile([O, 2], F32)
            nc.sync.dma_start(out=gb_sb[:], in_=gb[:])
            ident_sb = singles.tile([128, 128], F32)
            nc.sync.dma_start(out=ident_sb[:], in_=ident[:])

            xsT_aug = singles.tile([C + 1, 128], F32)  # row C is constant 1.0
            nc.vector.memset(xsT_aug[C : C + 1, :], 1.0)
            zero_t = singles.tile([128, 1], F32)
            nc.vector.memset(zero_t[:], 0.0)
            eps_t = singles.tile([O, 1], F32)
            nc.vector.memset(eps_t[:], BN_EPS)

            ybuf = singles.tile([128, T * 128], F32)
            obuf = singles.tile([128, T * 128], F16)
            sums = singles.tile([O, T], F32)
            sqs = singles.tile([O, T], F32)

            for t in range(T):
                nv = min(128, fpc - t * 128)
                if nv <= 0:
                    break
                # gather cat rows: one indirect DMA per k (128 faces each)
                gd = gd_pool.tile([128, K1, C], F16)
                for k in range(K1):
                    col = t * K1 + k
                    nc.gpsimd.indirect_dma_start(
                        out=gd[:, k, :],
                        out_offset=None,
                        in_=fea_full[:],
                        in_offset=bass.IndirectOffsetOnAxis(
                            ap=idx_sb[:, col : col + 1], axis=0
                        ),
                    )
                cat = cat_pool.tile([128, K1, C], F32)
                nc.vector.tensor_copy(out=cat[:], in_=gd[:])

                # ---- G = (Wq^T Wk xs + Wq^T bk)/sqrt(dk), face-major ----
                xsT_psum = pst.tile([C, 128], F32, tag="pst")
                nc.tensor.transpose(xsT_psum[:], cat[:, 0, :], ident_sb[:])
                nc.scalar.activation(xsT_aug[0:C, :], xsT_psum[:], AF.Copy)
                gt_psum = pst.tile([C, 128], F32, tag="pst")
                nc.tensor.matmul(
                    gt_psum[:], lhsT=a_sb[:], rhs=xsT_aug[:], start=True, stop=True
                )
                gt_sb = mid.tile([C, 128], F32, tag="gt")
                nc.scalar.activation(gt_sb[:], gt_psum[:], AF.Copy)
                gf_psum = pgf.tile([128, C], F32)
                nc.tensor.transpose(gf_psum[:], gt_sb[:], ident_sb[0:C, 0:C])
                gf_sb = mid.tile([128, C], F32, tag="gf")
                nc.scalar.activation(gf_sb[:], gf_psum[:], AF.Copy)

                # ---- logits[f,k] = sum_c G[f,c] * cat[f,k,c] (pre-scaled) ----
                prod = prod_pool.tile([128, K1, C], F32)
                gf_b = gf_sb[:].unsqueeze(1).to_broadcast([128, K1, C])
                nc.vector.tensor_tensor(out=prod[:], in0=cat[:], in1=gf_b, op=ALU.mult)
                logits = small.tile([128, K1], F32, tag="logits")
                nc.vector.tensor_reduce(
                    out=logits[:], in_=prod[:], axis=mybir.AxisListType.X, op=ALU.add
                )

                # ---- softmax over k (logits are small; skip max-sub) ----
                attu = small.tile([128, K1], F32, tag="attu")
                ssum = small.tile([128, 1], F32, tag="ssum")
                nc.scalar.activation(
                    attu[:], logits[:], AF.Exp, bias=zero_t[:], accum_out=ssum[:]
                )
                rinv = small.tile([128, 1], F32, tag="rinv")
                nc.vector.reciprocal(rinv[:], ssum[:])
                att = small.tile([128, K1], F32, tag="att")
                nc.vector.tensor_scalar(
                    out=att[:], in0=attu[:], scalar1=rinv[:], scalar2=None, op0=ALU.mult
                )

                # ---- agg[f,c] = sum_k att[f,k] * cat[f,k,c] ----
                prod2 = prod2_pool.tile([128, K1, C], F32)
                att_b = att[:].unsqueeze(2).to_broadcast([128, K1, C])
                nc.vector.tensor_tensor(
                    out=prod2[:], in0=cat[:], in1=att_b, op=ALU.mult
                )
                agg = mid.tile([128, C], F32, tag="agg")
                nc.vector.tensor_reduce(
                    out=agg[:],
                    in_=prod2[:].rearrange("p k c -> p c k"),
                    axis=mybir.AxisListType.X,
                    op=ALU.add,
                )

                # ---- y = Wc @ agg  (channel-major via PE transpose) ----
                aggT_psum = pst.tile([C, 128], F32, tag="pst")
                nc.tensor.transpose(aggT_psum[:], agg[:], ident_sb[:])
                aggT_sb = mid.tile([C, 128], F32, tag="aggT")
                nc.scalar.activation(aggT_sb[:], aggT_psum[:], AF.Copy)
                y_psum = py.tile([O, 128], F32)
                nc.tensor.matmul(
                    y_psum[:], lhsT=wct_sb[:], rhs=aggT_sb[:], start=True, stop=True
                )

                # ---- stash y + BN partial sums ----
                nc.scalar.activation(
                    ybuf[:, t * 128 : t * 128 + nv],
                    y_psum[:, 0:nv],
                    AF.Copy,
                    accum_out=sums[:, t : t + 1],
                )
                sq_scr = sq_pool.tile([O, 128], F32)
                nc.scalar.activation(
                    sq_scr[:, 0:nv],
                    y_psum[:, 0:nv],
                    AF.Square,
                    bias=zero_t[:],
                    accum_out=sqs[:, t : t + 1],
                )

            # ---- global BN stats ----
            stats_l = small.tile([O, 2], F32, tag="stats")
            nc.vector.tensor_reduce(
                out=stats_l[:, 0:1], in_=sums[:], axis=mybir.AxisListType.X, op=ALU.add
            )
            nc.vector.tensor_reduce(
                out=stats_l[:, 1:2], in_=sqs[:], axis=mybir.AxisListType.X, op=ALU.add
            )
            gst = small.tile([O, 2], F32, tag="gst")
            nc.sync.dma_start(out=cc_sin[:], in_=stats_l[:])
            nc.gpsimd.collective_compute(
                "AllReduce",
                ALU.add,
                replica_groups=[list(range(ndev))],
                ins=[cc_sin[:]],
                outs=[cc_sout[:]],
            )
            nc.sync.dma_start(out=gst[:], in_=cc_sout[:])

            mean = small.tile([O, 1], F32, tag="mean")
            nc.vector.tensor_scalar_mul(mean[:], gst[:, 0:1], 1.0 / ntot)
            e2 = small.tile([O, 1], F32, tag="e2")
            nc.vector.tensor_scalar_mul(e2[:], gst[:, 1:2], 1.0 / ntot)
            negvar = small.tile([O, 1], F32, tag="negvar")
            nc.vector.scalar_tensor_tensor(
                out=negvar[:],
                in0=mean[:],
                scalar=mean[:],
                in1=e2[:],
                op0=ALU.mult,
                op1=ALU.subtract,
            )
            sd = small.tile([O, 1], F32, tag="sd")
            nc.scalar.activation(sd[:], negvar[:], AF.Sqrt, bias=eps_t[:], scale=-1.0)
            rstd = small.tile([O, 1], F32, tag="rstd")
            nc.vector.reciprocal(rstd[:], sd[:])
            scale_v = small.tile([O, 1], F32, tag="scale_v")
            nc.vector.tensor_tensor(
                out=scale_v[:], in0=rstd[:], in1=gb_sb[:, 0:1], op=ALU.mult
            )
            negshift = small.tile([O, 1], F32, tag="negshift")
            nc.vector.scalar_tensor_tensor(
                out=negshift[:],
                in0=mean[:],
                scalar=scale_v[:],
                in1=gb_sb[:, 1:2],
                op0=ALU.mult,
                op1=ALU.subtract,
            )
            shift = small.tile([O, 1], F32, tag="shift")
            nc.vector.tensor_scalar_mul(shift[:], negshift[:], -1.0)

            # ---- final: relu((y - mean) * rstd * gamma + beta), f16 out ----
            nc.scalar.activation(
                obuf[:, 0:fpc], ybuf[:, 0:fpc], AF.Relu, bias=shift[:], scale=scale_v[:]
            )
            nc.sync.dma_start(out=y_out[:], in_=obuf[:, 0:fpc])

    nc.compile()
    return nc


def _pack_idx(pool_half, ring_half, T=_T):
    """[fpc,1]+[fpc,K] int -> int16 [128, T*K1] with idx[p, t*K1+k] =
    cat_idx[t*128+p, k] (padded with zeros)."""
    ci = np.concatenate([pool_half[:, None], ring_half], axis=1)
    pad = T * 128 - ci.shape[0]
    if pad:
        ci = np.concatenate([ci, np.zeros((pad, K1), ci.dtype)], 0)
    return np.ascontiguousarray(
        ci.reshape(T, 128, K1).transpose(1, 0, 2).reshape(128, T * K1)
    ).astype(np.int16)


def _prep_weights(Wk, bk, Wq, bq, Wc, gamma, beta):
    Wk = np.asarray(Wk, np.float64)
    Wq = np.asarray(Wq, np.float64)
    bk = np.asarray(bk, np.float64)
    a_mat = (Wk.T @ Wq) / SQRT_DK                 # [c, j]
    u = (Wq.T @ bk) / SQRT_DK                     # [j]
    a_aug = np.concatenate([a_mat, u[None, :]], 0).astype(np.float32)  # [C+1, C]
    wct = np.ascontiguousarray(np.asarray(Wc, np.float32).T)           # [C, O]
    gb = np.stack(
        [np.asarray(gamma, np.float32), np.asarray(beta, np.float32)], axis=1
    ).copy()                                                            # [O, 2]
    ident = np.eye(128, dtype=np.float32)
    return a_aug, wct, gb, ident


class _Exec:
    """Cached PJRT executable for the 8-core SPMD kernel.

    Mirrors concourse.bass2jax.run_bass_via_pjrt's multi-core path, but the
    jitted callable is built once, and the donated output buffers are created
    on device (run_bass_via_pjrt uploads host zeros on every call — at the
    ~40 MB/s axon link that is pure loss)."""

    def __init__(self, nc, n_cores):
        import jax
        import jax.numpy as jnp
        from jax.experimental.shard_map import shard_map
        from jax.sharding import Mesh, NamedSharding, PartitionSpec
        from concourse.bass2jax import (
            _bass_exec_p,
            install_neuronx_cc_hook,
            partition_id_tensor,
        )

        install_neuronx_cc_hook()
        assert nc.dbg_addr is None

        partition_name = (
            nc.partition_id_tensor.name if nc.partition_id_tensor else None
        )
        in_names, out_names, out_avals = [], [], []
        for alloc in nc.m.functions[0].allocations:
            if not isinstance(alloc, mybir.MemoryLocationSet):
                continue
            name = alloc.memorylocations[0].name
            if alloc.kind == "ExternalInput":
                if name != partition_name:
                    in_names.append(name)
            elif alloc.kind == "ExternalOutput":
                out_names.append(name)
                shape = tuple(alloc.tensor_shape)
                dtype = mybir.dt.np(alloc.dtype)
                out_avals.append(jax.core.ShapedArray(shape, dtype))
        n_params = len(in_names)
        n_outs = len(out_avals)
        all_in_names = list(in_names) + list(out_names)
        if partition_name is not None:
            all_in_names.append(partition_name)

        def _body(*args):
            operands = list(args)
            if partition_name is not None:
                operands.append(partition_id_tensor())
            outs = _bass_exec_p.bind(
                *operands,
                out_avals=tuple(out_avals),
                in_names=tuple(all_in_names),
                out_names=tuple(out_names),
                lowering_input_output_aliases=(),
                sim_require_finite=True,
                sim_require_nnan=True,
                nc=nc,
            )
            return tuple(outs)

        devices = jax.devices()[:n_cores]
        assert len(devices) == n_cores
        mesh = Mesh(np.asarray(devices), ("core",))
        in_specs = (PartitionSpec("core"),) * (n_params + n_outs)
        out_specs = (PartitionSpec("core"),) * n_outs
        donate = tuple(range(n_params, n_params + n_outs))
        self._sharded = jax.jit(
            shard_map(
                _body, mesh=mesh, in_specs=in_specs, out_specs=out_specs,
                check_rep=False,
            ),
            donate_argnums=donate,
            keep_unused=True,
        )

        osh = NamedSharding(mesh, PartitionSpec("core"))
        zero_shapes = [
            (n_cores * a.shape[0], *a.shape[1:]) for a in out_avals
        ]
        zero_dtypes = [a.dtype for a in out_avals]

        def _mk_zeros():
            return tuple(
                jax.lax.with_sharding_constraint(jnp.zeros(s, d), osh)
                for s, d in zip(zero_shapes, zero_dtypes)
            )

        self._mk_zeros = jax.jit(_mk_zeros)
        self._in_names = in_names
        self._out_names = out_names
        self._out_avals = out_avals
        self._n_cores = n_cores

    def run(self, in_maps):
        n = self._n_cores
        concat_in = [
            np.concatenate([np.asarray(m[name]) for m in in_maps], axis=0)
            for name in self._in_names
        ]
        zeros = self._mk_zeros()
        out_arrs = self._sharded(*concat_in, *zeros)
        return [
            {
                name: np.asarray(out_arrs[i]).reshape(
                    n, *self._out_avals[i].shape
                )[c]
                for i, name in enumerate(self._out_names)
            }
            for c in range(n)
        ]


_EXEC = None


def _get_exec():
    global _EXEC
    if _EXEC is None:
        nc = build_nc()
        _EXEC = _Exec(nc, NCORES)
    return _EXEC


def kernel(fea, ring_n, pool_idx, pos_embed=None, Wk=None, bk=None, Wq=None,
           bq=None, Wc=None, bc=None, gamma=None, beta=None):
    fea = np.asarray(fea, np.float32)
    ring_n = np.asarray(ring_n)
    pool_idx = np.asarray(pool_idx)
    fpc = _FPC

    a_aug, wct, gb, ident = _prep_weights(Wk, bk, Wq, bq, Wc, gamma, beta)
    # per-batch [F, C] f16 gather tables, split into halves along the face dim
    fea_t16 = np.ascontiguousarray(fea.transpose(0, 2, 1)).astype(np.float16)

    in_maps = []
    for c in range(NCORES):
        b, h = c // 2, c % 2
        in_maps.append(
            {
                "fea_h": fea_t16[b, h * FP : (h + 1) * FP],
                "idx16": _pack_idx(
                    pool_idx[h * fpc : (h + 1) * fpc],
                    ring_n[b, h * fpc : (h + 1) * fpc],
                ),
                "a_aug": a_aug,
                "wct": wct,
                "gb": gb,
                "ident": ident,
            }
        )

    res = _get_exec().run(in_maps)

    out = np.empty((B, O, FP), np.float32)
    for c in range(NCORES):
        b, h = c // 2, c % 2
        out[b, :, h * fpc : (h + 1) * fpc] = res[c]["y_out"].astype(np.float32)
    return out
